# revision 23
# baseline (speedup 1.0000x reference)
"""FP8 block-quantized MoE MLP (16 experts, top-4 routing) on 8 Trainium2 cores.

Strategy (expert-parallel):
  Host: build routing tables from top_k_index; each core owns 2 experts.
    The reference's dynamic per-token/per-128-group fp8-e4m3fn activation
    quant-dequant is bit-exact elementwise, so it is applied once on the
    host (ml_dtypes e4m3fn == jnp.float8_e4m3fn), cast to fp16, and the
    gathered rows are laid out PRE-TRANSPOSED per 128-row tile so the
    device consumes them directly as matmul stationary operands.
    Weights are dequantized in f32 and cast to fp16 (fp16 keeps the
    end-to-end rel-err ~5e-3, well under the 2e-2 gate; bf16 would not).
  Device (per core, per 128-row tile):
    GEMM1 (8 K-blocks, stationary = pre-transposed activations), then
    silu(gate)*up, fp8 re-quantization (hardware fp8e4 cast with a /2
    exponent shift: TRN fp8e4 max-normal is 240 vs OCP 448), PE-transpose
    of the quantized intermediate, GEMM2, scale rows by the routing
    weight, DMA the fp16 rows out.
  Host: scatter-add the weighted rows into the [2048, 1024] f32 output.

  DMA is split across three queues ordered by first-use (w1[e0] | w2/w1[e1]
  | x tiles) so the tensor engine is never weight-starved; total HBM
  traffic is ~10 MB/core vs 19 MB for the f32-weight variant.
"""

import numpy as np
import ml_dtypes

# Problem constants (hardcoded per the task contract).
T = 2048
H = 1024
I_DIM = 512
E = 16
TK = 4
G = 128
FP8_MAX = 448.0
NCORES = 8
E_LOC = E // NCORES          # experts per core
KB1 = H // 128               # 8 contraction blocks for gate_up
KB2 = I_DIM // 128           # 4 contraction blocks for down
O1 = 2 * I_DIM               # 1024
O2 = H                       # 1024
OB1 = O1 // G                # 8 output scale blocks for gate_up

_PROGRAM_CACHE: dict = {}


def _build_program(cap: int, do_compile: bool = True):
    import concourse.bass as bass
    import concourse.mybir as mybir
    from concourse import bacc
    from concourse.tile import TileContext
    from concourse.masks import make_identity
    from contextlib import ExitStack

    dt = mybir.dt
    F32 = dt.float32
    FP8 = dt.float8e4
    F16 = dt.float16
    R = E_LOC * cap
    ntiles = R // 128
    tpe = cap // 128

    nc = bacc.Bacc("TRN2")
    # xq: dequantized activations, tile-major pre-transposed:
    #   xq[j, t, kb, r] = x_deq[t*128 + r, kb*128 + j]
    xq_d = nc.dram_tensor("xq", [128, ntiles, KB1, 128], F16, kind="ExternalInput")
    rw_d = nc.dram_tensor("rw", [128, ntiles], F32, kind="ExternalInput")
    # w1 ships as halved fp8 codes + a per-(128x128)-block scale grid
    # replicated across partitions; the exact fp16 dequant (code/2 * 2s)
    # happens on the DVE. This halves the start-gating weight stream.
    w1q_d = nc.dram_tensor("w1q", [E_LOC, 128, KB1, O1], FP8, kind="ExternalInput")
    w1s_d = nc.dram_tensor("w1s", [128, E_LOC, KB1, OB1], F32, kind="ExternalInput")
    w2_d = nc.dram_tensor("w2", [E_LOC, 128, KB2, O2], F16, kind="ExternalInput")
    out_d = nc.dram_tensor("out", [R, H], F16, kind="ExternalOutput")

    AX = mybir.AxisListType.X
    OP = mybir.AluOpType
    ACT = mybir.ActivationFunctionType

    def bcast(ap, reps):
        # [P, g] -> [P, g, reps] with a stride-0 innermost dim
        return bass.AP(tensor=ap.tensor, offset=ap.offset,
                       ap=[ap.ap[0], ap.ap[1], [0, reps]])

    def bcast_col(ap, reps):
        # [P, 1] -> [P, reps] with a stride-0 free dim
        return bass.AP(tensor=ap.tensor, offset=ap.offset,
                       ap=[ap.ap[0], [0, reps]])

    with TileContext(nc) as tc, ExitStack() as ctx:
        singles = ctx.enter_context(tc.tile_pool(name="singles", bufs=1))
        spool = ctx.enter_context(tc.tile_pool(name="spool", bufs=8))
        hpool = ctx.enter_context(tc.tile_pool(name="hpool", bufs=2))
        qpool = ctx.enter_context(tc.tile_pool(name="qpool", bufs=2))
        dqpool = ctx.enter_context(tc.tile_pool(name="dqpool", bufs=2))
        tpool = ctx.enter_context(tc.tile_pool(name="tpool", bufs=2))
        opool = ctx.enter_context(tc.tile_pool(name="opool", bufs=3))
        ps_gu = ctx.enter_context(tc.tile_pool(name="ps_gu", bufs=2, space="PSUM"))
        ps_tx = ctx.enter_context(tc.tile_pool(name="ps_tx", bufs=1, space="PSUM"))
        ps_o = ctx.enter_context(tc.tile_pool(name="ps_o", bufs=1, space="PSUM"))

        # Warmup source on the (idle until tile 0's h-phase) vector queue so
        # the first PE instruction doesn't wait on gpsimd's DMA issues; the
        # real identity (first needed by tile 0's h-transpose, ~15us in)
        # stays on gpsimd ahead of its DMA issues.
        warm_src = singles.tile([128, 128], F16)
        nc.vector.memset(warm_src, 0.0)
        ident = singles.tile([128, 128], F16)
        make_identity(nc, ident)

        w1q_sb = singles.tile([128, E_LOC, KB1, O1], FP8)
        w1s_sb = singles.tile([128, E_LOC, KB1, OB1], F32)
        w1_sb = singles.tile([128, E_LOC, KB1, O1], F16)
        w2_sb = singles.tile([128, E_LOC, KB2, O2], F16)
        xq_sb = singles.tile([128, ntiles, KB1, 128], F16)
        rw_sb = singles.tile([128, ntiles], F32)

        # DMA, ordered by first use and split across queues so the first
        # GEMM can start early and later experts stream in behind. HBM
        # bandwidth is shared across queues, so the critical first-tile
        # data (w1 codes on sync; xq0 leading gpsimd) stays in front.
        nc.sync.dma_start(out=w1s_sb, in_=w1s_d[:, :])
        for e in range(E_LOC):
            for kb in range(KB1):
                nc.sync.dma_start(out=w1q_sb[:, e, kb], in_=w1q_d[e, :, kb])
        for it in range(min(2, ntiles)):
            nc.gpsimd.dma_start(out=xq_sb[:, it], in_=xq_d[:, it])
        nc.gpsimd.dma_start(out=rw_sb, in_=rw_d[:, :])
        for kb in range(KB2):
            nc.gpsimd.dma_start(out=w2_sb[:, 0, kb], in_=w2_d[0, :, kb])
        for it in range(2, ntiles):
            nc.gpsimd.dma_start(out=xq_sb[:, it], in_=xq_d[:, it])
        for e in range(1, E_LOC):
            for kb in range(KB2):
                nc.gpsimd.dma_start(out=w2_sb[:, e, kb], in_=w2_d[e, :, kb])

        def emit_w1_dequant(e, kb):
            nc.vector.tensor_tensor(
                out=w1_sb[:, e, kb].rearrange("p (ob oi) -> p ob oi", oi=128),
                in0=w1q_sb[:, e, kb].rearrange("p (ob oi) -> p ob oi", oi=128),
                in1=bcast(w1s_sb[:, e, kb], 128), op=OP.mult)

        # expert 0's dequant must precede the h-chains on the DVE queue;
        # expert 1's is spread over early iterations (it has until ~t4).
        for kb in range(KB1):
            emit_w1_dequant(0, kb)

        # PE warmup: opens the HAM clock gate / p-state ramp while the
        # first weight chunks land (first DMA completion lags ~4.5us).
        warm = ps_tx.tile([128, 4, 128], F16, tag="ps_tx")
        for j in range(64):
            nc.tensor.transpose(warm[:, j % 4], warm_src, warm_src)

        state = {}

        def emit_g1(it):
            """GEMM1 for tile `it`: stationary = pre-transposed x block."""
            e = it // tpe
            pg = ps_gu.tile([128, 1024], F32, tag="ps_gu")
            for kb in range(KB1):
                nc.tensor.matmul(pg[:, 0:512], xq_sb[:, it, kb],
                                 w1_sb[:, e, kb, 0:512],
                                 start=(kb == 0), stop=(kb == KB1 - 1))
                nc.tensor.matmul(pg[:, 512:1024], xq_sb[:, it, kb],
                                 w1_sb[:, e, kb, 512:1024],
                                 start=(kb == 0), stop=(kb == KB1 - 1))
            state[it] = pg

        def emit_h(it):
            """silu(gate)*up + fp8 re-quantization (DVE/ACT work only)."""
            pg = state[it]
            h_t = hpool.tile([128, I_DIM], F32, tag="h")
            nc.scalar.activation(out=h_t, in_=pg[:, 0:512], func=ACT.Silu)
            nc.vector.tensor_tensor(out=h_t, in0=h_t, in1=pg[:, 512:1024],
                                    op=OP.mult)
            amax_h = spool.tile([128, KB2], F32, tag="amax_h")
            nc.vector.tensor_reduce(
                out=amax_h, in_=h_t.rearrange("p (g j) -> p g j", j=128),
                axis=AX, op=OP.max, apply_absolute_value=True)
            scl2h = spool.tile([128, KB2], F32, tag="scl_h")
            nc.vector.tensor_scalar(out=scl2h, in0=amax_h, scalar1=1e-10,
                                    scalar2=2.0 / FP8_MAX, op0=OP.max,
                                    op1=OP.mult)
            inv2h = spool.tile([128, KB2], F32, tag="inv_h")
            nc.vector.reciprocal(inv2h, scl2h)
            q8h = qpool.tile([128, KB2, 128], FP8, tag="q8_h")
            nc.vector.tensor_tensor(
                out=q8h, in0=h_t.rearrange("p (g j) -> p g j", j=128),
                in1=bcast(inv2h, 128), op=OP.mult)
            hq = dqpool.tile([128, KB2, 128], F16, tag="hq")
            nc.vector.tensor_tensor(out=hq, in0=q8h, in1=bcast(scl2h, 128),
                                    op=OP.mult)
            state[it] = {"hq": hq}

        def emit_txg2(it):
            """transpose + GEMM2 + weighted fp16 output (PE + copies)."""
            e = it // tpe
            r0 = it * 128
            hq = state.pop(it)["hq"]
            psh = ps_tx.tile([128, 4, 128], F16, tag="ps_tx")
            for kb in range(KB2):
                nc.tensor.transpose(psh[:, kb], hq[:, kb, :], ident)
            hT = tpool.tile([128, KB2, 128], F16, tag="hT")
            nc.scalar.copy(out=hT, in_=psh)
            # GEMM2 half-by-half so each half's scaled copy + DMA-out runs
            # under the other half's matmuls (shrinks the end-of-kernel
            # tail). The halves use SEPARATE psum tiles: hazards are
            # tracked per-tile, so a shared tile would stall half1's
            # matmuls on half0's copy. copy0 runs on scalar, copy1 on
            # vector (per-partition rw scale via a stride-0 broadcast) to
            # split the copy load across engines.
            o_t = opool.tile([128, H], F16, tag="o")
            for half in range(2):
                c0 = half * 512
                po = ps_o.tile([128, 512], F32, tag=f"ps_o{half}")
                for kb in range(KB2):
                    nc.tensor.matmul(po, hT[:, kb, :],
                                     w2_sb[:, e, kb, c0:c0 + 512],
                                     start=(kb == 0), stop=(kb == KB2 - 1))
                if half == 0:
                    nc.scalar.activation(out=o_t[:, c0:c0 + 512], in_=po,
                                         func=ACT.Copy,
                                         scale=rw_sb[:, it:it + 1])
                else:
                    nc.vector.tensor_tensor(
                        out=o_t[:, c0:c0 + 512], in0=po,
                        in1=bcast_col(rw_sb[:, it:it + 1], 512), op=OP.mult)
                nc.sync.dma_start(out=out_d[r0:r0 + 128, c0:c0 + 512],
                                  in_=o_t[:, c0:c0 + 512])

        # depth-2 software pipeline: the PE runs g1(t+1) tx(t-1) g2(t-1),
        # so the DVE quant chain of tile t has a full extra tile of slack
        # before the PE consumes hq(t).
        for it in range(ntiles):
            emit_g1(it)
            if it >= 1:
                emit_h(it - 1)
            if it in (1, 2):
                for kb in range(4 * (it - 1), 4 * it):
                    emit_w1_dequant(1, kb)
            if it >= 2:
                emit_txg2(it - 2)
        if ntiles >= 2:
            emit_txg2(ntiles - 2)
        emit_h(ntiles - 1)
        emit_txg2(ntiles - 1)

    if do_compile:
        nc.compile()
    return nc


def _get_program(cap: int):
    if cap not in _PROGRAM_CACHE:
        _PROGRAM_CACHE[cap] = _build_program(cap)
    return _PROGRAM_CACHE[cap]


def _dequant_weight(w, s, g=G):
    E_, O_, K_ = w.shape
    wb = w.reshape(E_, O_ // g, g, K_ // g, g)
    return (wb * s[:, :, None, :, None]).reshape(E_, O_, K_)


def _qdq_act_fp16(x):
    """Exact e4m3fn per-token/per-128-group quant-dequant, then fp16."""
    Tn, K = x.shape
    xg = x.reshape(Tn, K // G, G)
    amax = np.max(np.abs(xg), axis=-1, keepdims=True)
    scale = np.maximum(amax, 1e-10) / FP8_MAX
    q = np.clip(xg / scale, -FP8_MAX, FP8_MAX)
    q = q.astype(ml_dtypes.float8_e4m3fn).astype(np.float32)
    return (q * scale).reshape(Tn, K).astype(np.float16)


def _prep(inputs):
    hs = np.ascontiguousarray(np.asarray(inputs["hidden_states"], np.float32))
    idx = np.asarray(inputs["top_k_index"]).astype(np.int64)
    tkw = np.asarray(inputs["top_k_weights"], np.float32)
    gup = np.asarray(inputs["gate_up_proj"], np.float32)
    gup_s = np.asarray(inputs["gate_up_proj_scale_inv"], np.float32)
    dn = np.asarray(inputs["down_proj"], np.float32)
    dn_s = np.asarray(inputs["down_proj_scale_inv"], np.float32)

    # routing tables: merge duplicate (token, expert) pairs (the reference
    # sums top-k weights per expert), then group by expert
    flat_e = idx.reshape(-1)
    flat_t = np.repeat(np.arange(T, dtype=np.int64), TK)
    flat_w = tkw.reshape(-1).astype(np.float64)
    key = flat_e * T + flat_t
    uk, inv = np.unique(key, return_inverse=True)
    sw = np.bincount(inv, weights=flat_w).astype(np.float32)
    se = (uk // T).astype(np.int64)
    st = (uk % T).astype(np.int64)
    counts = np.bincount(se, minlength=E)
    cap = int(np.ceil(max(int(counts.max()), 1) / 128.0) * 128)
    R = E_LOC * cap
    ntiles = R // 128

    starts = np.zeros(E + 1, np.int64)
    np.cumsum(counts, out=starts[1:])

    # activations: exact fp8 qdq once on the host, fp16
    xdq = _qdq_act_fp16(hs)                          # [T, H] fp16

    # w1 ships as halved fp8 codes (TRN e4m3 max-normal 240 vs the codes'
    # e4m3fn 448) plus a doubled scale grid replicated over partitions;
    # the device DVE computes fp16(code/2 * 2s) == fp16(code * s).
    w1q_t = np.ascontiguousarray(
        (gup * 0.5).transpose(0, 2, 1).reshape(E, KB1, 128, O1)
        .transpose(0, 2, 1, 3)
    ).astype(ml_dtypes.float8_e4m3)                  # [E, 128, KB1, O1]
    w1s = (2.0 * gup_s).transpose(0, 2, 1)           # [E, KB1, OB1]
    w2_full = _dequant_weight(dn, dn_s)              # [E, O2, I]
    w2_t = np.ascontiguousarray(
        w2_full.transpose(0, 2, 1).reshape(E, KB2, 128, O2).transpose(0, 2, 1, 3)
    ).astype(np.float16)

    in_maps = []
    tok_core = []      # per-core valid token ids (concatenated per expert)
    nvalid_core = []   # per-core list of (row_offset, count)
    for c in range(NCORES):
        rows_idx = np.zeros(R, np.int64)
        rw_vec = np.zeros(R, np.float32)
        segs = []
        for j in range(E_LOC):
            e = c * E_LOC + j
            n = int(counts[e])
            s0, r0 = starts[e], j * cap
            rows_idx[r0:r0 + n] = st[s0:s0 + n]
            rw_vec[r0:r0 + n] = sw[s0:s0 + n]
            segs.append((r0, n))
        xg = xdq[rows_idx]                           # [R, H] fp16
        # tile-major pre-transpose: [j, tile, kb, row-in-tile]
        xqt = np.ascontiguousarray(
            xg.reshape(ntiles, 128, KB1, 128).transpose(3, 0, 2, 1))
        w1s_rep = np.ascontiguousarray(np.broadcast_to(
            w1s[c * E_LOC:(c + 1) * E_LOC][None],
            (128, E_LOC, KB1, OB1)).astype(np.float32))
        in_maps.append({
            "xq": xqt,
            "rw": np.ascontiguousarray(rw_vec.reshape(ntiles, 128).T),
            "w1q": np.ascontiguousarray(w1q_t[c * E_LOC:(c + 1) * E_LOC]),
            "w1s": w1s_rep,
            "w2": np.ascontiguousarray(w2_t[c * E_LOC:(c + 1) * E_LOC]),
        })
        tok_core.append(rows_idx)
        nvalid_core.append(segs)
    return cap, in_maps, tok_core, nvalid_core


def _combine(results, tok_core, nvalid_core):
    out = np.zeros((T, H), np.float32)
    for c in range(NCORES):
        res = results[c]["out"].astype(np.float32)
        for (r0, n) in nvalid_core[c]:
            if n:
                np.add.at(out, tok_core[c][r0:r0 + n], res[r0:r0 + n])
    return out


def kernel_with_results(inputs, trace=False):
    from concourse.bass_utils import run_bass_kernel_spmd
    cap, in_maps, tok_core, nvalid_core = _prep(inputs)
    nc = _get_program(cap)
    bres = run_bass_kernel_spmd(nc, in_maps, core_ids=list(range(NCORES)),
                                trace=trace)
    out = _combine(bres.results, tok_core, nvalid_core)
    return out, bres


def kernel(**inputs) -> np.ndarray:
    out, _ = kernel_with_results(inputs, trace=False)
    return out


# revision 28
# speedup vs baseline: 1.2382x; 1.2382x over previous
"""FP8 block-quantized MoE MLP (16 experts, top-4 routing) on 8 Trainium2 cores.

Strategy (expert-parallel):
  Host: build routing tables from top_k_index; each core owns 2 experts.
    The reference's dynamic per-token/per-128-group fp8-e4m3fn activation
    quant-dequant is bit-exact elementwise, so it is applied once on the
    host (ml_dtypes e4m3fn == jnp.float8_e4m3fn), cast to fp16, and the
    gathered rows are laid out PRE-TRANSPOSED per 128-row tile so the
    device consumes them directly as matmul stationary operands.
    Weights are dequantized in f32 and cast to fp16 (fp16 keeps the
    end-to-end rel-err ~5e-3, well under the 2e-2 gate; bf16 would not).
  Device (per core, per 128-row tile):
    GEMM1 (8 K-blocks, stationary = pre-transposed activations), then
    silu(gate)*up, fp8 re-quantization (hardware fp8e4 cast with a /2
    exponent shift: TRN fp8e4 max-normal is 240 vs OCP 448), PE-transpose
    of the quantized intermediate, GEMM2, scale rows by the routing
    weight, DMA the fp16 rows out.
  Host: scatter-add the weighted rows into the [2048, 1024] f32 output.

  DMA is split across three queues ordered by first-use (w1[e0] | w2/w1[e1]
  | x tiles) so the tensor engine is never weight-starved; total HBM
  traffic is ~10 MB/core vs 19 MB for the f32-weight variant.
"""

import numpy as np
import ml_dtypes

# Problem constants (hardcoded per the task contract).
T = 2048
H = 1024
I_DIM = 512
E = 16
TK = 4
G = 128
FP8_MAX = 448.0
NCORES = 8
E_LOC = E // NCORES          # experts per core
KB1 = H // 128               # 8 contraction blocks for gate_up
KB2 = I_DIM // 128           # 4 contraction blocks for down
O1 = 2 * I_DIM               # 1024
O2 = H                       # 1024
OB1 = O1 // G                # 8 output scale blocks for gate_up

_PROGRAM_CACHE: dict = {}


def _build_program(cap: int, do_compile: bool = True):
    import concourse.bass as bass
    import concourse.mybir as mybir
    from concourse import bacc
    from concourse.tile import TileContext
    from concourse.masks import make_identity
    from contextlib import ExitStack

    dt = mybir.dt
    F32 = dt.float32
    FP8 = dt.float8e4
    F16 = dt.float16
    R = E_LOC * cap
    ntiles = R // 128
    tpe = cap // 128

    nc = bacc.Bacc("TRN2")
    # xq: dequantized activations, tile-major pre-transposed:
    #   xq[j, t, kb, r] = x_deq[t*128 + r, kb*128 + j]
    xq_d = nc.dram_tensor("xq", [128, ntiles, KB1, 128], F16, kind="ExternalInput")
    rw_d = nc.dram_tensor("rw", [128, ntiles], F32, kind="ExternalInput")
    w1_d = nc.dram_tensor("w1", [E_LOC, 128, KB1, O1], F16, kind="ExternalInput")
    w2_d = nc.dram_tensor("w2", [E_LOC, 128, KB2, O2], F16, kind="ExternalInput")
    out_d = nc.dram_tensor("out", [R, H], F16, kind="ExternalOutput")

    AX = mybir.AxisListType.X
    OP = mybir.AluOpType
    ACT = mybir.ActivationFunctionType

    def bcast(ap, reps):
        # [P, g] -> [P, g, reps] with a stride-0 innermost dim
        return bass.AP(tensor=ap.tensor, offset=ap.offset,
                       ap=[ap.ap[0], ap.ap[1], [0, reps]])

    def bcast_col(ap, reps):
        # [P, 1] -> [P, reps] with a stride-0 free dim
        return bass.AP(tensor=ap.tensor, offset=ap.offset,
                       ap=[ap.ap[0], [0, reps]])

    with TileContext(nc) as tc, ExitStack() as ctx:
        singles = ctx.enter_context(tc.tile_pool(name="singles", bufs=1))
        spool = ctx.enter_context(tc.tile_pool(name="spool", bufs=8))
        hpool = ctx.enter_context(tc.tile_pool(name="hpool", bufs=2))
        qpool = ctx.enter_context(tc.tile_pool(name="qpool", bufs=2))
        dqpool = ctx.enter_context(tc.tile_pool(name="dqpool", bufs=2))
        tpool = ctx.enter_context(tc.tile_pool(name="tpool", bufs=2))
        opool = ctx.enter_context(tc.tile_pool(name="opool", bufs=3))
        ps_gu = ctx.enter_context(tc.tile_pool(name="ps_gu", bufs=2, space="PSUM"))
        ps_tx = ctx.enter_context(tc.tile_pool(name="ps_tx", bufs=1, space="PSUM"))
        ps_o = ctx.enter_context(tc.tile_pool(name="ps_o", bufs=1, space="PSUM"))

        # Warmup source on the (idle until tile 0's h-phase) vector queue so
        # the first PE instruction doesn't wait on gpsimd's DMA issues; the
        # real identity (first needed by tile 0's h-transpose, ~15us in)
        # stays on gpsimd ahead of its DMA issues.
        warm_src = singles.tile([128, 128], F16)
        nc.vector.memset(warm_src, 0.0)
        ident = singles.tile([128, 128], F16)
        make_identity(nc, ident)

        w1_sb = singles.tile([128, E_LOC, KB1, O1], F16)
        w2_sb = singles.tile([128, E_LOC, KB2, O2], F16)
        xq_sb = singles.tile([128, ntiles, KB1, 128], F16)
        rw_sb = singles.tile([128, ntiles], F32)

        # DMA, ordered by first use. w1 (the start-gating stream) is split
        # across BOTH hw-DGE queues (sync even kbs / scalar odd kbs) so
        # expert 0 lands in half the time; everything needed later rides
        # gpsimd (sw-DGE, laggy but off the critical window).
        for e in range(E_LOC):
            for kb in range(0, KB1, 2):
                nc.sync.dma_start(out=w1_sb[:, e, kb], in_=w1_d[e, :, kb])
        for e in range(E_LOC):
            for kb in range(1, KB1, 2):
                nc.scalar.dma_start(out=w1_sb[:, e, kb], in_=w1_d[e, :, kb])
        for it in range(min(2, ntiles)):
            nc.gpsimd.dma_start(out=xq_sb[:, it], in_=xq_d[:, it])
        nc.gpsimd.dma_start(out=rw_sb, in_=rw_d[:, :])
        for kb in range(KB2):
            nc.gpsimd.dma_start(out=w2_sb[:, 0, kb], in_=w2_d[0, :, kb])
        for it in range(2, ntiles):
            nc.gpsimd.dma_start(out=xq_sb[:, it], in_=xq_d[:, it])
        for e in range(1, E_LOC):
            for kb in range(KB2):
                nc.gpsimd.dma_start(out=w2_sb[:, e, kb], in_=w2_d[e, :, kb])

        # PE warmup: opens the HAM clock gate / p-state ramp while the
        # first weight chunks land (first DMA completion lags ~4.5us).
        warm = ps_tx.tile([128, 4, 128], F16, tag="ps_tx")
        for j in range(64):
            nc.tensor.transpose(warm[:, j % 4], warm_src, warm_src)

        state = {}

        def emit_g1(it):
            """GEMM1 for tile `it`: stationary = pre-transposed x block."""
            e = it // tpe
            pg = ps_gu.tile([128, 1024], F32, tag="ps_gu")
            for kb in range(KB1):
                nc.tensor.matmul(pg[:, 0:512], xq_sb[:, it, kb],
                                 w1_sb[:, e, kb, 0:512],
                                 start=(kb == 0), stop=(kb == KB1 - 1))
                nc.tensor.matmul(pg[:, 512:1024], xq_sb[:, it, kb],
                                 w1_sb[:, e, kb, 512:1024],
                                 start=(kb == 0), stop=(kb == KB1 - 1))
            state[it] = pg

        def emit_h(it):
            """silu(gate)*up + fp8 re-quantization (DVE/ACT work only)."""
            pg = state[it]
            h_t = hpool.tile([128, I_DIM], F32, tag="h")
            nc.scalar.activation(out=h_t, in_=pg[:, 0:512], func=ACT.Silu)
            nc.vector.tensor_tensor(out=h_t, in0=h_t, in1=pg[:, 512:1024],
                                    op=OP.mult)
            amax_h = spool.tile([128, KB2], F32, tag="amax_h")
            nc.vector.tensor_reduce(
                out=amax_h, in_=h_t.rearrange("p (g j) -> p g j", j=128),
                axis=AX, op=OP.max, apply_absolute_value=True)
            scl2h = spool.tile([128, KB2], F32, tag="scl_h")
            nc.vector.tensor_scalar(out=scl2h, in0=amax_h, scalar1=1e-10,
                                    scalar2=2.0 / FP8_MAX, op0=OP.max,
                                    op1=OP.mult)
            inv2h = spool.tile([128, KB2], F32, tag="inv_h")
            nc.vector.reciprocal(inv2h, scl2h)
            q8h = qpool.tile([128, KB2, 128], FP8, tag="q8_h")
            nc.vector.tensor_tensor(
                out=q8h, in0=h_t.rearrange("p (g j) -> p g j", j=128),
                in1=bcast(inv2h, 128), op=OP.mult)
            hq = dqpool.tile([128, KB2, 128], F16, tag="hq")
            nc.vector.tensor_tensor(out=hq, in0=q8h, in1=bcast(scl2h, 128),
                                    op=OP.mult)
            state[it] = {"hq": hq}

        def emit_txg2(it):
            """transpose + GEMM2 + weighted fp16 output (PE + copies)."""
            e = it // tpe
            r0 = it * 128
            hq = state.pop(it)["hq"]
            psh = ps_tx.tile([128, 4, 128], F16, tag="ps_tx")
            for kb in range(KB2):
                nc.tensor.transpose(psh[:, kb], hq[:, kb, :], ident)
            hT = tpool.tile([128, KB2, 128], F16, tag="hT")
            nc.scalar.copy(out=hT, in_=psh)
            # GEMM2 half-by-half so each half's scaled copy + DMA-out runs
            # under the other half's matmuls (shrinks the end-of-kernel
            # tail). The halves use SEPARATE psum tiles: hazards are
            # tracked per-tile, so a shared tile would stall half1's
            # matmuls on half0's copy. copy0 runs on scalar, copy1 on
            # vector (per-partition rw scale via a stride-0 broadcast) to
            # split the copy load across engines.
            o_t = opool.tile([128, H], F16, tag="o")
            for half in range(2):
                c0 = half * 512
                po = ps_o.tile([128, 512], F32, tag=f"ps_o{half}")
                for kb in range(KB2):
                    nc.tensor.matmul(po, hT[:, kb, :],
                                     w2_sb[:, e, kb, c0:c0 + 512],
                                     start=(kb == 0), stop=(kb == KB2 - 1))
                if half == 0:
                    nc.scalar.activation(out=o_t[:, c0:c0 + 512], in_=po,
                                         func=ACT.Copy,
                                         scale=rw_sb[:, it:it + 1])
                else:
                    nc.vector.tensor_tensor(
                        out=o_t[:, c0:c0 + 512], in0=po,
                        in1=bcast_col(rw_sb[:, it:it + 1], 512), op=OP.mult)
                nc.sync.dma_start(out=out_d[r0:r0 + 128, c0:c0 + 512],
                                  in_=o_t[:, c0:c0 + 512])

        # depth-2 software pipeline: the PE runs g1(t+1) tx(t-1) g2(t-1),
        # so the DVE quant chain of tile t has a full extra tile of slack
        # before the PE consumes hq(t).
        for it in range(ntiles):
            emit_g1(it)
            if it >= 1:
                emit_h(it - 1)
            if it >= 2:
                emit_txg2(it - 2)
        if ntiles >= 2:
            emit_txg2(ntiles - 2)
        emit_h(ntiles - 1)
        emit_txg2(ntiles - 1)

    if do_compile:
        nc.compile()
    return nc


def _get_program(cap: int):
    if cap not in _PROGRAM_CACHE:
        _PROGRAM_CACHE[cap] = _build_program(cap)
    return _PROGRAM_CACHE[cap]


def _dequant_weight(w, s, g=G):
    E_, O_, K_ = w.shape
    wb = w.reshape(E_, O_ // g, g, K_ // g, g)
    return (wb * s[:, :, None, :, None]).reshape(E_, O_, K_)


def _qdq_act_fp16(x):
    """Exact e4m3fn per-token/per-128-group quant-dequant, then fp16."""
    Tn, K = x.shape
    xg = x.reshape(Tn, K // G, G)
    amax = np.max(np.abs(xg), axis=-1, keepdims=True)
    scale = np.maximum(amax, 1e-10) / FP8_MAX
    q = np.clip(xg / scale, -FP8_MAX, FP8_MAX)
    q = q.astype(ml_dtypes.float8_e4m3fn).astype(np.float32)
    return (q * scale).reshape(Tn, K).astype(np.float16)


def _prep(inputs):
    hs = np.ascontiguousarray(np.asarray(inputs["hidden_states"], np.float32))
    idx = np.asarray(inputs["top_k_index"]).astype(np.int64)
    tkw = np.asarray(inputs["top_k_weights"], np.float32)
    gup = np.asarray(inputs["gate_up_proj"], np.float32)
    gup_s = np.asarray(inputs["gate_up_proj_scale_inv"], np.float32)
    dn = np.asarray(inputs["down_proj"], np.float32)
    dn_s = np.asarray(inputs["down_proj_scale_inv"], np.float32)

    # routing tables: merge duplicate (token, expert) pairs (the reference
    # sums top-k weights per expert), then group by expert
    flat_e = idx.reshape(-1)
    flat_t = np.repeat(np.arange(T, dtype=np.int64), TK)
    flat_w = tkw.reshape(-1).astype(np.float64)
    key = flat_e * T + flat_t
    uk, inv = np.unique(key, return_inverse=True)
    sw = np.bincount(inv, weights=flat_w).astype(np.float32)
    se = (uk // T).astype(np.int64)
    st = (uk % T).astype(np.int64)
    counts = np.bincount(se, minlength=E)
    cap = int(np.ceil(max(int(counts.max()), 1) / 128.0) * 128)
    R = E_LOC * cap
    ntiles = R // 128

    starts = np.zeros(E + 1, np.int64)
    np.cumsum(counts, out=starts[1:])

    # activations: exact fp8 qdq once on the host, fp16
    xdq = _qdq_act_fp16(hs)                          # [T, H] fp16

    # weights: exact f32 dequant, [K-block, O]-transposed, fp16
    w1_full = _dequant_weight(gup, gup_s)            # [E, O1, H]
    w1_t = np.ascontiguousarray(
        w1_full.transpose(0, 2, 1).reshape(E, KB1, 128, O1).transpose(0, 2, 1, 3)
    ).astype(np.float16)
    w2_full = _dequant_weight(dn, dn_s)              # [E, O2, I]
    w2_t = np.ascontiguousarray(
        w2_full.transpose(0, 2, 1).reshape(E, KB2, 128, O2).transpose(0, 2, 1, 3)
    ).astype(np.float16)

    in_maps = []
    tok_core = []      # per-core valid token ids (concatenated per expert)
    nvalid_core = []   # per-core list of (row_offset, count)
    for c in range(NCORES):
        rows_idx = np.zeros(R, np.int64)
        rw_vec = np.zeros(R, np.float32)
        segs = []
        for j in range(E_LOC):
            e = c * E_LOC + j
            n = int(counts[e])
            s0, r0 = starts[e], j * cap
            rows_idx[r0:r0 + n] = st[s0:s0 + n]
            rw_vec[r0:r0 + n] = sw[s0:s0 + n]
            segs.append((r0, n))
        xg = xdq[rows_idx]                           # [R, H] fp16
        # tile-major pre-transpose: [j, tile, kb, row-in-tile]
        xqt = np.ascontiguousarray(
            xg.reshape(ntiles, 128, KB1, 128).transpose(3, 0, 2, 1))
        in_maps.append({
            "xq": xqt,
            "rw": np.ascontiguousarray(rw_vec.reshape(ntiles, 128).T),
            "w1": np.ascontiguousarray(w1_t[c * E_LOC:(c + 1) * E_LOC]),
            "w2": np.ascontiguousarray(w2_t[c * E_LOC:(c + 1) * E_LOC]),
        })
        tok_core.append(rows_idx)
        nvalid_core.append(segs)
    return cap, in_maps, tok_core, nvalid_core


def _combine(results, tok_core, nvalid_core):
    out = np.zeros((T, H), np.float32)
    for c in range(NCORES):
        res = results[c]["out"].astype(np.float32)
        for (r0, n) in nvalid_core[c]:
            if n:
                np.add.at(out, tok_core[c][r0:r0 + n], res[r0:r0 + n])
    return out


def kernel_with_results(inputs, trace=False):
    from concourse.bass_utils import run_bass_kernel_spmd
    cap, in_maps, tok_core, nvalid_core = _prep(inputs)
    nc = _get_program(cap)
    bres = run_bass_kernel_spmd(nc, in_maps, core_ids=list(range(NCORES)),
                                trace=trace)
    out = _combine(bres.results, tok_core, nvalid_core)
    return out, bres


def kernel(**inputs) -> np.ndarray:
    out, _ = kernel_with_results(inputs, trace=False)
    return out
